# revision 1
# baseline (speedup 1.0000x reference)
"""Trainium2 Bass kernel for nn_BatchGeneralization (scatter_memory).

ret = x;  ret[ref_index] = x[target_index] * mag + x[ref_index] * (1 - mag)

Strategy (8-core SPMD, data-parallel over the batch dim):
  - Assign the ~819 ref rows round-robin to cores (<=103 each), pad to
    MAXM=128 mix slots per core. Permute each core's 1024-row shard so its
    mix rows occupy positions [0, MAXM).
  - Host gathers the matching target rows (x[target_index]) per core, plus
    per-row mag / (1-mag) columns.
  - Device kernel per core (identical instruction stream, per-core data):
      * pass-through rows: DRAM->DRAM DMA copies, split across both HWDGE
        rings (ACT carries most + the mix-row store mid-stream; SP carries
        the mix-path SBUF loads first, then the remaining rows)
      * mix rows: load to SBUF, blend o = xs*(1-m) + tg*m on DVE, store
  - Host scatters each core's rows back into the full output.

The split ratio (P_ACT pass-through rows on the ACT ring, ACT_PRE of them
before the mix store slot) was tuned on hardware; both HWDGE rings sustain
~250 GB/s one-way each on DRAM->DRAM, ~400-600 GB/s aggregate (HBM-pair
bound), so the whole kernel is DMA-roofline limited.
"""

import sys

for _p in ("/opt/trn_rl_repo", "/root/.axon_site/_ro/trn_rl_repo"):
    if _p not in sys.path:
        sys.path.append(_p)

import numpy as np

import concourse.bass as bass
from concourse import mybir
from concourse.bass_utils import run_bass_kernel_spmd

N_CORES = 8
B, D = 8192, 4096
R = B // N_CORES   # rows per core
MAXM = 128         # mix slots per core (>= ceil(819/8) = 103)
P_ACT = 680        # pass-through rows on the ACT ring
ACT_PRE = 144      # of those, rows copied before the mix-store slot

_NC = None


def _build_nc():
    nc = bass.Bass("TRN2", debug=False)
    f32 = mybir.dt.float32

    xs = nc.dram_tensor("xs", [R, D], f32, kind="ExternalInput").ap()
    tg = nc.dram_tensor("tg", [MAXM, D], f32, kind="ExternalInput").ap()
    mg = nc.dram_tensor("mg", [MAXM, 1], f32, kind="ExternalInput").ap()
    om = nc.dram_tensor("om", [MAXM, 1], f32, kind="ExternalInput").ap()
    out_mix = nc.dram_tensor("out_mix", [MAXM, D], f32, kind="ExternalOutput").ap()
    out_rest = nc.dram_tensor("out_rest", [R - MAXM, D], f32, kind="ExternalOutput").ap()

    a_sb = nc.alloc_sbuf_tensor("a_sb", [MAXM, D], f32).ap()
    b_sb = nc.alloc_sbuf_tensor("b_sb", [MAXM, D], f32).ap()
    t_sb = nc.alloc_sbuf_tensor("t_sb", [MAXM, D], f32).ap()
    o_sb = nc.alloc_sbuf_tensor("o_sb", [MAXM, D], f32).ap()
    m_sb = nc.alloc_sbuf_tensor("m_sb", [MAXM, 1], f32).ap()
    w_sb = nc.alloc_sbuf_tensor("w_sb", [MAXM, 1], f32).ap()

    with (
        nc.Block() as block,
        nc.semaphore("s_in") as s_in,
        nc.semaphore("s_big") as s_big,
        nc.semaphore("s_out") as s_out,
        nc.semaphore("s_ve") as s_ve,
    ):
        # ACT ring: bulk copy with the mix-row store slotted mid-stream
        @block.scalar
        def _(scalar):
            scalar.dma_start(
                out=out_rest[0:ACT_PRE, :], in_=xs[MAXM:MAXM + ACT_PRE, :]
            ).then_inc(s_big, 16)
            scalar.wait_ge(s_ve, 1)
            scalar.dma_start(out=out_mix, in_=o_sb).then_inc(s_out, 16)
            scalar.dma_start(
                out=out_rest[ACT_PRE:P_ACT, :], in_=xs[MAXM + ACT_PRE:MAXM + P_ACT, :]
            ).then_inc(s_big, 16)
            scalar.wait_ge(s_big, 32)
            scalar.wait_ge(s_out, 16)

        # SP ring: mix-path loads first, then the remaining bulk rows
        @block.sync
        def _(sync):
            sync.dma_start(out=m_sb, in_=mg).then_inc(s_in, 16)
            sync.dma_start(out=w_sb, in_=om).then_inc(s_in, 16)
            sync.dma_start(out=b_sb, in_=tg).then_inc(s_in, 16)
            sync.dma_start(out=a_sb, in_=xs[0:MAXM, :]).then_inc(s_in, 16)
            sync.dma_start(
                out=out_rest[P_ACT:, :], in_=xs[MAXM + P_ACT:R, :]
            ).then_inc(s_big, 16)
            sync.wait_ge(s_big, 48)

        # DVE: o = xs*(1-m) + tg*m, matching the reference fp ordering.
        # t = tg*m only needs the first three loads (ring completions are
        # FIFO), so start it before the xs mix rows land.
        @block.vector
        def _(vector):
            vector.wait_ge(s_in, 48)
            vector.tensor_scalar_mul(t_sb, b_sb, m_sb)
            vector.wait_ge(s_in, 64)
            vector.scalar_tensor_tensor(
                o_sb, a_sb, w_sb, t_sb,
                mybir.AluOpType.mult, mybir.AluOpType.add,
            ).then_inc(s_ve, 1)

    return nc


def _get_nc():
    global _NC
    if _NC is None:
        _NC = _build_nc()
    return _NC


def _prepare(x, ref_index, target_index, mag):
    """Build per-core input maps + the row assignment for unsharding."""
    x = np.ascontiguousarray(np.asarray(x, dtype=np.float32))
    ref = np.asarray(ref_index).astype(np.int64).ravel()
    tgt = np.asarray(target_index).astype(np.int64).ravel()
    mag = np.asarray(mag, dtype=np.float32).ravel()
    n_mix = ref.shape[0]

    # keep only the LAST occurrence of each ref row (sequential last-write-wins)
    _, rev_idx = np.unique(ref[::-1], return_index=True)
    keep = np.sort(n_mix - 1 - rev_idx)
    ref_u, tgt_u, mag_u = ref[keep], np.clip(tgt[keep], 0, B - 1), mag[keep]
    nm = ref_u.shape[0]

    is_ref = np.zeros(B, dtype=bool)
    is_ref[ref_u] = True
    nonref = np.nonzero(~is_ref)[0]

    in_maps = []
    rows_list = []
    pos = 0
    for c in range(N_CORES):
        sel = np.arange(c, nm, N_CORES)
        n_c = sel.shape[0]
        assert n_c <= MAXM, f"core {c}: {n_c} ref rows > {MAXM} slots"
        n_fill = R - n_c
        fill = nonref[pos:pos + n_fill]
        pos += n_fill
        rows = np.concatenate([ref_u[sel], fill])
        rows_list.append(rows)

        mg_c = np.zeros((MAXM, 1), dtype=np.float32)
        mg_c[:n_c, 0] = mag_u[sel]
        om_c = 1.0 - mg_c
        tg_c = np.zeros((MAXM, D), dtype=np.float32)
        tg_c[:n_c] = x[tgt_u[sel]]

        in_maps.append({
            "xs": x[rows],
            "tg": tg_c,
            "mg": mg_c,
            "om": om_c,
        })
    return in_maps, rows_list


def _run(in_maps, rows_list, **kwargs):
    nc = _get_nc()
    res = run_bass_kernel_spmd(nc, in_maps, list(range(N_CORES)), **kwargs)
    out = np.empty((B, D), dtype=np.float32)
    for c in range(N_CORES):
        rows = rows_list[c]
        out[rows[:MAXM]] = res.results[c]["out_mix"]
        out[rows[MAXM:]] = res.results[c]["out_rest"]
    return out, res


def kernel(x, y, ref_index, target_index, mag):
    in_maps, rows_list = _prepare(x, ref_index, target_index, mag)
    out, _ = _run(in_maps, rows_list)
    return out


def kernel_profiled(x, y, ref_index, target_index, mag, **trace_kwargs):
    """Same as kernel() but runs with NTFF tracing; returns (out, results)."""
    in_maps, rows_list = _prepare(x, ref_index, target_index, mag)
    out, res = _run(in_maps, rows_list, trace=True, **trace_kwargs)
    return out, res

